# revision 21
# baseline (speedup 1.0000x reference)
"""Trainium2 Bass kernel for nn_Attention_85169201480311.

Dense transformer block: 3x (conv3x3 -> GroupNorm(1) -> exact GELU) projections,
8-head attention over 1024 tokens with relative-position bias, 1x1 out-conv.

Sharding: data-parallel over batch (8 samples -> 8 cores), params replicated.

Per-core program layout notes:
 - conv3x3 = 18 PSUM-accumulating matmuls (2 cin chunks x 9 taps) against a
   zero-padded [128, 2, 34, 34] SBUF image.
 - GroupNorm stats: bn_stats/bn_aggr per psum tile + gpsimd partition_all_reduce;
   affine+GELU fused into the PSUM eviction via ACT scale/bias operands.
 - attention in scores^T layout (m on partitions): both matmuls consume the
   native conv output layout. Softmax without max-subtraction; relative-position
   bias applied multiplicatively post-exp: exp(s*S)*exp(bias), with exp(bias)
   precomputed on host (bf16) and streamed.
 - attn@v packs 4 heads per PSUM tile via tile_position col packing; row-sums
   from separate M=1 ones-matmuls; normalization deferred to the [128,1024]
   group output using a reciprocal row broadcast via SBUF->SBUF DMA.
"""
import sys
for p in ('/opt/trn_rl_repo', '/root/.axon_site/_ro/trn_rl_repo'):
    if p not in sys.path:
        sys.path.insert(0, p)

import numpy as np
import ml_dtypes

import concourse.bass as bass
import concourse.tile as tile
from concourse import mybir, bacc, bass_isa
from concourse import bass_utils

F32 = mybir.dt.float32
BF16 = mybir.dt.bfloat16
AF = mybir.ActivationFunctionType

IH = IW = 32
N = IH * IW          # 1024 tokens
C = 256
HEADS = 8
DH = 32
SCALE = 32 ** -0.5
EPS = 1e-6
B = 8
P = 128
NCHUNK = C // P      # 2 channel chunks

_cache = {}
DEBUG_TAPS = False


def _rel_index():
    coords = np.stack(np.meshgrid(np.arange(IH), np.arange(IW), indexing='ij')).reshape(2, -1)
    rel = coords[:, :, None] - coords[:, None, :]
    rel[0] += IH - 1
    rel[1] += IW - 1
    rel[0] *= 2 * IW - 1
    return rel.sum(0)  # [n, m] int


def build_nc():
    nc = bacc.Bacc('TRN2', target_bir_lowering=False)

    x_d = nc.dram_tensor("x", [NCHUNK, P, IH, IW], F32, kind="ExternalInput")
    w_d = {}
    for nm in ("wq", "wk", "wv"):
        w_d[nm] = nc.dram_tensor(nm, [P, NCHUNK, 9, C], F32, kind="ExternalInput")
    wout_d = nc.dram_tensor("wout", [P, NCHUNK, C], F32, kind="ExternalInput")
    vecs_d = nc.dram_tensor("vecs", [P, 14], F32, kind="ExternalInput")
    eb_d = nc.dram_tensor("ebias", [HEADS, 8, P, N], BF16, kind="ExternalInput")
    out_d = nc.dram_tensor("out", [P, NCHUNK, N], F32, kind="ExternalOutput")
    dbg = {}
    if DEBUG_TAPS:
        for nm, shp, dt in (("dbg_q", [P, NCHUNK, N], F32), ("dbg_k", [P, NCHUNK, N], F32),
                            ("dbg_v", [P, NCHUNK, N], BF16), ("dbg_vtr", [P, 8, C], BF16),
                            ("dbg_st", [P, 3, 4], F32), ("dbg_ao", [P, NCHUNK, N], F32)):
            dbg[nm] = nc.dram_tensor(nm, shp, dt, kind="ExternalOutput")

    # vecs cols: gq0,gq1,bq0,bq1, gk0,gk1,bk0,bk1, gv0,gv1,bv0,bv1, bout0,bout1
    VGCOL = {"wq": 0, "wk": 4, "wv": 8}

    with tile.TileContext(nc) as tc:
        with tc.tile_pool(name="const", bufs=1) as const, \
             tc.tile_pool(name="proj", bufs=1) as proj, \
             tc.tile_pool(name="stats", bufs=2) as stats_p, \
             tc.tile_pool(name="attn", bufs=1) as attn_p:

            vecs = const.tile([P, 14], F32)
            nc.sync.dma_start(vecs[:], vecs_d[:])
            eps_t = const.tile([P, 1], F32)
            nc.vector.memset(eps_t[:], EPS)
            ones_bf = const.tile([P, 1], BF16)
            nc.vector.memset(ones_bf[:], 1.0)
            wout_sb = const.tile([P, NCHUNK, C], F32)
            nc.sync.dma_start(wout_sb[:], wout_d[:])

            xpad = const.tile([P, NCHUNK, IH + 2, IW + 2], F32)
            nc.gpsimd.memset(xpad[:], 0.0)
            for c in range(NCHUNK):
                nc.sync.dma_start(xpad[:, c, 1:IH + 1, 1:IW + 1], x_d[c])

            q_sb = proj.tile([P, NCHUNK, N], F32)
            k_sb = proj.tile([P, NCHUNK, N], F32)
            v_bf = proj.tile([P, NCHUNK, N], BF16)
            v_tr = proj.tile([P, 8, C], BF16)
            attn_out = attn_p.tile([P, NCHUNK, N], F32)
            out_sb = attn_p.tile([P, NCHUNK, N], F32)

            # ---------------- projections ----------------
            with tc.tile_pool(name="wpool", bufs=1) as wpool, \
                 tc.tile_pool(name="psc", bufs=8, space="PSUM") as psc:
                w_sb = {}
                for nm in ("wq", "wk", "wv"):
                    w_sb[nm] = wpool.tile([P, NCHUNK, 9, C], F32, name=f"sb_{nm}")
                    nc.sync.dma_start(w_sb[nm][:], w_d[nm][:])

                for iproj, (nm, dst, odt) in enumerate(
                        (("wq", q_sb, F32), ("wk", k_sb, F32), ("wv", v_bf, BF16))):
                    ps_t = [psc.tile([P, 512], F32, name=f"cv_{nm}_{m}_{j}", tag="conv")
                            for m in range(2) for j in range(2)]
                    st_t = [stats_p.tile([P, 2, 6], F32, name=f"st_{nm}_{m}", tag="stats")
                            for m in range(2)]
                    for m in range(2):
                        for j in range(2):
                            pt = ps_t[m * 2 + j]
                            first = True
                            for c in range(NCHUNK):
                                for t in range(9):
                                    dy, dx = t // 3, t % 3
                                    rhs = xpad[:, c, 16 * j + dy:16 * j + dy + 16, dx:dx + 32]
                                    nc.tensor.matmul(
                                        pt[:], w_sb[nm][:, c, t, m * P:(m + 1) * P], rhs,
                                        start=first, stop=(c == NCHUNK - 1 and t == 8))
                                    first = False
                            nc.vector.bn_stats(out=st_t[m][:, j, :], in_=pt[:])
                    # per-channel mean/var over the full 1024 spatial
                    mv = [stats_p.tile([P, 2], F32, name=f"mv_{nm}_{m}", tag="mv") for m in range(2)]
                    for m in range(2):
                        nc.vector.bn_aggr(out=mv[m][:], in_=st_t[m][:])
                    prep = stats_p.tile([P, 4], F32, name=f"prep_{nm}", tag="prep")
                    for m in range(2):
                        nc.vector.tensor_copy(out=prep[:, 2 * m:2 * m + 1], in_=mv[m][:, 0:1])
                        sq = stats_p.tile([P, 1], F32, name=f"sq_{nm}_{m}", tag="sq")
                        nc.vector.tensor_mul(out=sq[:], in0=mv[m][:, 0:1], in1=mv[m][:, 0:1])
                        nc.vector.tensor_add(out=prep[:, 2 * m + 1:2 * m + 2],
                                             in0=mv[m][:, 1:2], in1=sq[:])
                    red = stats_p.tile([P, 4], F32, name=f"red_{nm}", tag="red")
                    nc.gpsimd.partition_all_reduce(red[:], prep[:], channels=P,
                                                   reduce_op=bass_isa.ReduceOp.add)
                    # mean = (c0+c2)/256 ; var = (c1+c3)/256 - mean^2
                    mt = stats_p.tile([P, 4], F32, name=f"mt_{nm}", tag="mt")
                    nc.vector.tensor_add(out=mt[:, 0:1], in0=red[:, 0:1], in1=red[:, 2:3])
                    nc.scalar.mul(out=mt[:, 0:1], in_=mt[:, 0:1], mul=1.0 / C)
                    nc.vector.tensor_add(out=mt[:, 1:2], in0=red[:, 1:2], in1=red[:, 3:4])
                    nc.scalar.mul(out=mt[:, 1:2], in_=mt[:, 1:2], mul=1.0 / C)
                    nc.vector.tensor_mul(out=mt[:, 2:3], in0=mt[:, 0:1], in1=mt[:, 0:1])
                    nc.vector.tensor_sub(out=mt[:, 1:2], in0=mt[:, 1:2], in1=mt[:, 2:3])
                    # rstd = 1/sqrt(var + eps)
                    nc.scalar.activation(out=mt[:, 1:2], in_=mt[:, 1:2], func=AF.Sqrt,
                                         bias=eps_t[:], scale=1.0)
                    nc.vector.reciprocal(out=mt[:, 1:2], in_=mt[:, 1:2])
                    if DEBUG_TAPS:
                        nc.sync.dma_start(dbg["dbg_st"][:, iproj, :], mt[:])
                    gc = VGCOL[nm]
                    sc = stats_p.tile([P, 4], F32, name=f"sc_{nm}", tag="sc")
                    for m in range(2):
                        # s_m = g_m * rstd ; t_m = b_m - mean * s_m
                        nc.vector.tensor_mul(out=sc[:, m:m + 1],
                                             in0=vecs[:, gc + m:gc + m + 1], in1=mt[:, 1:2])
                        nc.vector.tensor_mul(out=sc[:, 2 + m:3 + m],
                                             in0=mt[:, 0:1], in1=sc[:, m:m + 1])
                        nc.vector.tensor_sub(out=sc[:, 2 + m:3 + m],
                                             in0=vecs[:, gc + 2 + m:gc + 3 + m],
                                             in1=sc[:, 2 + m:3 + m])
                    for m in range(2):
                        for j in range(2):
                            nc.scalar.activation(
                                out=dst[:, m, 512 * j:512 * (j + 1)], in_=ps_t[m * 2 + j][:],
                                func=AF.Gelu, scale=sc[:, m:m + 1], bias=sc[:, 2 + m:3 + m])

            if DEBUG_TAPS:
                nc.sync.dma_start(dbg["dbg_q"][:], q_sb[:])
                nc.sync.dma_start(dbg["dbg_k"][:], k_sb[:])
                nc.sync.dma_start(dbg["dbg_v"][:], v_bf[:])

            # v transpose: [c, n] -> [n, c] via bf16 DMA transpose, 16 x [128,128]
            for k in range(NCHUNK):
                for i in range(8):
                    nc.sync.dma_start_transpose(
                        v_tr[:, i, P * k:P * (k + 1)], v_bf[:, k, P * i:P * (i + 1)])

            if DEBUG_TAPS:
                nc.sync.dma_start(dbg["dbg_vtr"][:], v_tr[:])

            # ---------------- attention ----------------
            from contextlib import ExitStack
            with ExitStack() as stk:
                attnT_p = stk.enter_context(tc.tile_pool(name="attnT_p", bufs=4))
                ebuf = stk.enter_context(tc.tile_pool(name="ebuf", bufs=4))
                ebias_p = stk.enter_context(tc.tile_pool(name="ebias_p", bufs=4))
                rc_p = stk.enter_context(tc.tile_pool(name="rc", bufs=2))
                ps_sc = stk.enter_context(tc.tile_pool(name="ps_sc", bufs=2, space="PSUM"))
                ps_avp = stk.enter_context(tc.tile_pool(name="ps_avp", bufs=1, space="PSUM"))
                ps_rsp = stk.enter_context(tc.tile_pool(name="ps_rsp", bufs=1, space="PSUM"))
                ps_av = ps_rs = None
                for pair in range(4):
                    grp = pair // 2
                    if pair % 2 == 0:
                        ps_av = ps_avp.tile([P, N], F32, name=f"av_{grp}", tag="av")
                        ps_rs = ps_rsp.tile([P, N], F32, name=f"rs_{grp}", tag="rs")
                    for h in (2 * pair, 2 * pair + 1):
                        g, r = h // 4, h % 4
                        attnT = attnT_p.tile([P, 8, N], BF16, name=f"attnT_{h}", tag="attnT")
                        for i in range(8):
                            ps_s = ps_sc.tile([P, N], F32, name=f"s_{h}_{i}", tag="sc")
                            for nj in range(2):
                                nc.tensor.matmul(
                                    ps_s[:, 512 * nj:512 * (nj + 1)],
                                    k_sb[32 * r:32 * r + 32, g, P * i:P * (i + 1)],
                                    q_sb[32 * r:32 * r + 32, g, 512 * nj:512 * (nj + 1)],
                                    start=True, stop=True, tile_position=(32 * r, 0))
                            e_bf = ebuf.tile([P, N], BF16, name=f"e_{h}_{i}", tag="e")
                            nc.scalar.activation(out=e_bf[:], in_=ps_s[:], func=AF.Exp,
                                                 scale=SCALE)
                            eb_t = ebias_p.tile([P, N], BF16, name=f"eb_{h}_{i}", tag="eb")
                            nc.sync.dma_start(eb_t[:], eb_d[h, i])
                            nc.vector.tensor_mul(out=attnT[:, i, :], in0=e_bf[:], in1=eb_t[:])
                        # attn @ v for this head (+ row-sums), accumulating into the
                        # group psum at col position 32*r
                        for nj in range(2):
                            for i in range(8):
                                nc.tensor.matmul(
                                    ps_av[32 * r:32 * r + 32, 512 * nj:512 * (nj + 1)],
                                    v_tr[:, i, 32 * h:32 * h + 32],
                                    attnT[:, i, 512 * nj:512 * (nj + 1)],
                                    start=(i == 0), stop=(i == 7), tile_position=(0, 32 * r))
                            for i in range(8):
                                nc.tensor.matmul(
                                    ps_rs[32 * r:32 * r + 1, 512 * nj:512 * (nj + 1)],
                                    ones_bf[:, 0:1],
                                    attnT[:, i, 512 * nj:512 * (nj + 1)],
                                    start=(i == 0), stop=(i == 7), tile_position=(0, 32 * r))
                    if pair % 2 == 1:
                        rcp = rc_p.tile([P, N], F32, name=f"rcp_{grp}", tag="rcp")
                        rcp_bc = rc_p.tile([P, N], F32, name=f"rcpbc_{grp}", tag="rcpbc")
                        for r in range(4):
                            nc.vector.reciprocal(out=rcp[32 * r:32 * r + 1, :],
                                                 in_=ps_rs[32 * r:32 * r + 1, :])
                            row = rcp[32 * r:32 * r + 1, :]
                            src = bass.AP(tensor=row.tensor, offset=row.offset,
                                          ap=[list(row.ap[0]), [0, 32]]
                                          + [list(d) for d in row.ap[1:]])
                            nc.sync.dma_start(out=rcp_bc[32 * r:32 * r + 32, :], in_=src)
                        nc.vector.tensor_mul(out=attn_out[:, grp, :], in0=ps_av[:],
                                             in1=rcp_bc[:])

                if DEBUG_TAPS:
                    nc.sync.dma_start(dbg["dbg_ao"][:], attn_out[:])

                # ---------------- 1x1 out conv ----------------
                for m in range(2):
                    for j in range(2):
                        ps_o = ps_sc.tile([P, 512], F32, name=f"o_{m}_{j}", tag="sc")
                        for k in range(NCHUNK):
                            nc.tensor.matmul(ps_o[:], wout_sb[:, k, m * P:(m + 1) * P],
                                             attn_out[:, k, 512 * j:512 * (j + 1)],
                                             start=(k == 0), stop=(k == NCHUNK - 1))
                        nc.scalar.activation(out=out_sb[:, m, 512 * j:512 * (j + 1)],
                                             in_=ps_o[:], func=AF.Identity,
                                             bias=vecs[:, 12 + m:13 + m], scale=1.0)
                nc.sync.dma_start(out_d[:], out_sb[:])

    nc.compile()
    return nc


def _prep_shared(Wq, gq, bq, Wk, gk, bk, Wv, gv, bv, bias_table, Wout, bout):
    def wt(W):
        # [co, ci, 3, 3] -> [ci%128, ci//128, tap, co]
        w = np.ascontiguousarray(W.transpose(1, 2, 3, 0).reshape(NCHUNK, P, 9, C)
                                 .transpose(1, 0, 2, 3)).astype(np.float32)
        return w
    vecs = np.zeros((P, 14), np.float32)
    for col, v in ((0, gq), (2, bq), (4, gk), (6, bk), (8, gv), (10, bv), (12, bout)):
        vecs[:, col] = v[:P]
        vecs[:, col + 1] = v[P:]
    wout = np.ascontiguousarray(Wout[:, :, 0, 0].T.reshape(NCHUNK, P, C)
                                .transpose(1, 0, 2)).astype(np.float32)
    idx = _rel_index()                     # [n, m]
    eb = np.exp(bias_table.astype(np.float32))[idx]   # [n, m, H]
    ebT = eb.transpose(2, 1, 0)            # [H, m, n]
    ebias = ebT.reshape(HEADS, 8, P, N).astype(ml_dtypes.bfloat16)
    return {"wq": wt(Wq), "wk": wt(Wk), "wv": wt(Wv), "vecs": vecs,
            "wout": wout, "ebias": np.ascontiguousarray(ebias)}


def kernel(x, Wq, gq, bq, Wk, gk, bk, Wv, gv, bv, bias_table, Wout, bout):
    x = np.asarray(x, np.float32)
    if "nc" not in _cache:
        _cache["nc"] = build_nc()
    nc = _cache["nc"]
    shared = _prep_shared(np.asarray(Wq), np.asarray(gq), np.asarray(bq),
                          np.asarray(Wk), np.asarray(gk), np.asarray(bk),
                          np.asarray(Wv), np.asarray(gv), np.asarray(bv),
                          np.asarray(bias_table), np.asarray(Wout), np.asarray(bout))
    in_maps = []
    for b in range(B):
        m = dict(shared)
        m["x"] = np.ascontiguousarray(x[b].reshape(NCHUNK, P, IH, IW))
        in_maps.append(m)
    _cache["last_in_maps"] = in_maps
    res = bass_utils.run_bass_kernel_spmd(nc, in_maps, core_ids=list(range(B)))
    out = np.stack([r["out"] for r in res.results])          # [B, 128, 2, 1024]
    out = out.transpose(0, 2, 1, 3).reshape(B, C, IH, IW)
    return np.ascontiguousarray(out.astype(np.float32))


if __name__ == "__main__":
    rng = np.random.default_rng(0)
    inputs = {
        'x': rng.standard_normal((B, C, IH, IW), dtype=np.float32),
        'Wq': (rng.standard_normal((C, C, 3, 3)) * 0.02).astype(np.float32),
        'gq': np.ones(C, np.float32), 'bq': np.zeros(C, np.float32),
        'Wk': (rng.standard_normal((C, C, 3, 3)) * 0.02).astype(np.float32),
        'gk': np.ones(C, np.float32), 'bk': np.zeros(C, np.float32),
        'Wv': (rng.standard_normal((C, C, 3, 3)) * 0.02).astype(np.float32),
        'gv': np.ones(C, np.float32), 'bv': np.zeros(C, np.float32),
        'bias_table': (rng.standard_normal(((2 * IH - 1) * (2 * IW - 1), HEADS)) * 0.02).astype(np.float32),
        'Wout': (rng.standard_normal((C, C, 1, 1)) * 0.02).astype(np.float32),
        'bout': np.zeros(C, np.float32),
    }
    out = kernel(**inputs)
    print("out", out.shape, out.dtype, np.abs(out).max())


# revision 24
# speedup vs baseline: 2.0271x; 2.0271x over previous
"""Trainium2 Bass kernel for nn_Attention_85169201480311.

Dense transformer block: 3x (conv3x3 -> GroupNorm(1) -> exact GELU) projections,
8-head attention over 1024 tokens with relative-position bias, 1x1 out-conv.

Sharding: data-parallel over batch (8 samples -> 8 cores), params replicated.

Per-core program:
 - conv3x3 = 18 PSUM-accumulating bf16 matmuls (2 cin chunks x 9 taps) against a
   zero-padded [128, 2, 34, 34] SBUF image.
 - GroupNorm stats: bn_stats/bn_aggr per psum tile + gpsimd partition_all_reduce;
   affine+GELU fused into the PSUM eviction via ACT scale/bias operands.
 - attention in scores^T layout (m on partitions): both matmuls consume the
   native conv output layout, K=32 scores matmuls row-packed 4-up via
   tile_position. Softmax without max-subtraction; relative-position bias
   applied multiplicatively post-exp: exp(s*S)*exp(bias), exp(bias) precomputed
   host-side (bf16) and streamed.
 - attn@v: lhsT = [v_head | ones] (M=33), so psum row 32/96 carries the softmax
   denominator; two heads per psum tile via tile_position col packing. The
   reciprocal runs on a [128,16] respread of the sums (DMA repartition), and the
   normalizing multiply reads psum directly, writing the partition-aligned
   [c, n] layout the 1x1 out-conv consumes.
"""
import sys
for p in ('/opt/trn_rl_repo', '/root/.axon_site/_ro/trn_rl_repo'):
    if p not in sys.path:
        sys.path.insert(0, p)

import numpy as np
import ml_dtypes

import concourse.bass as bass
import concourse.tile as tile
from concourse import mybir, bacc, bass_isa
from concourse import bass_utils
from concourse.masks import make_identity

F32 = mybir.dt.float32
BF16 = mybir.dt.bfloat16
AF = mybir.ActivationFunctionType

IH = IW = 32
N = IH * IW          # 1024 tokens
C = 256
HEADS = 8
DH = 32
SCALE = 32 ** -0.5
EPS = 1e-6
B = 8
P = 128
NCHUNK = C // P      # 2 channel chunks

_cache = {}
DEBUG_TAPS = False


def _rel_index():
    coords = np.stack(np.meshgrid(np.arange(IH), np.arange(IW), indexing='ij')).reshape(2, -1)
    rel = coords[:, :, None] - coords[:, None, :]
    rel[0] += IH - 1
    rel[1] += IW - 1
    rel[0] *= 2 * IW - 1
    return rel.sum(0)  # [n, m] int


def build_nc():
    nc = bacc.Bacc('TRN2', target_bir_lowering=False)

    x_d = nc.dram_tensor("x", [NCHUNK, P, IH, IW], BF16, kind="ExternalInput")
    w_d = {}
    for nm in ("wq", "wk", "wv"):
        w_d[nm] = nc.dram_tensor(nm, [P, NCHUNK, 9, C], BF16, kind="ExternalInput")
    wout_d = nc.dram_tensor("wout", [P, NCHUNK, C], F32, kind="ExternalInput")
    vecs_d = nc.dram_tensor("vecs", [P, 14], F32, kind="ExternalInput")
    eb_d = nc.dram_tensor("ebias", [HEADS, 8, P, N], BF16, kind="ExternalInput")
    out_d = nc.dram_tensor("out", [P, NCHUNK, N], F32, kind="ExternalOutput")
    dbg = {}
    if DEBUG_TAPS:
        for nm, shp, dt in (("dbg_q", [P, NCHUNK, N], BF16), ("dbg_k", [P, NCHUNK, N], BF16),
                            ("dbg_v", [P, NCHUNK, N], BF16), ("dbg_va", [P, 8, 8, 34], BF16),
                            ("dbg_st", [P, 3, 4], F32), ("dbg_ao", [P, NCHUNK, N], F32)):
            dbg[nm] = nc.dram_tensor(nm, shp, dt, kind="ExternalOutput")

    # vecs cols: gq0,gq1,bq0,bq1, gk0,gk1,bk0,bk1, gv0,gv1,bv0,bv1, bout0,bout1
    VGCOL = {"wq": 0, "wk": 4, "wv": 8}

    with tile.TileContext(nc) as tc:
        with tc.tile_pool(name="const", bufs=1) as const, \
             tc.tile_pool(name="proj", bufs=1) as proj, \
             tc.tile_pool(name="stats", bufs=2) as stats_p, \
             tc.tile_pool(name="attn", bufs=1) as attn_p, \
             tc.tile_pool(name="ps_sc", bufs=2, space="PSUM") as ps_sc:

            vecs = const.tile([P, 14], F32)
            nc.sync.dma_start(vecs[:], vecs_d[:])
            eps_t = const.tile([P, 1], F32)
            nc.vector.memset(eps_t[:], EPS)
            wout_sb = const.tile([P, NCHUNK, C], F32)
            nc.sync.dma_start(wout_sb[:], wout_d[:])
            ident = const.tile([P, P], BF16)
            make_identity(nc, ident[:])

            xpad = const.tile([P, NCHUNK, IH + 2, IW + 2], BF16)
            nc.gpsimd.memset(xpad[:], 0.0)
            for c in range(NCHUNK):
                nc.sync.dma_start(xpad[:, c, 1:IH + 1, 1:IW + 1], x_d[c])

            q_sb = proj.tile([P, NCHUNK, N], BF16)
            k_sb = proj.tile([P, NCHUNK, N], BF16)
            v_bf = proj.tile([P, NCHUNK, N], BF16)
            v_aug = proj.tile([P, 8, 8, 34], BF16)   # [*, i, h, 0:32 v | 32 ones | pad]
            attn_out = attn_p.tile([P, NCHUNK, N], F32)
            out_sb = attn_p.tile([P, NCHUNK, N], F32)

            # ---------------- projections ----------------
            with tc.tile_pool(name="wpool", bufs=1) as wpool, \
                 tc.tile_pool(name="psc", bufs=4, space="PSUM") as psc:
                w_sb = {}
                for nm in ("wq", "wk", "wv"):
                    w_sb[nm] = wpool.tile([P, NCHUNK, 9, C], BF16, name=f"sb_{nm}")
                    nc.sync.dma_start(w_sb[nm][:], w_d[nm][:])

                for iproj, (nm, dst) in enumerate(
                        (("wq", q_sb), ("wk", k_sb), ("wv", v_bf))):
                    ps_t = [psc.tile([P, 512], F32, name=f"cv_{nm}_{m}_{j}", tag="conv")
                            for m in range(2) for j in range(2)]
                    st_t = [stats_p.tile([P, 2, 6], F32, name=f"st_{nm}_{m}", tag="stats")
                            for m in range(2)]
                    for m in range(2):
                        for j in range(2):
                            pt = ps_t[m * 2 + j]
                            first = True
                            for c in range(NCHUNK):
                                for t in range(9):
                                    dy, dx = t // 3, t % 3
                                    rhs = xpad[:, c, 16 * j + dy:16 * j + dy + 16, dx:dx + 32]
                                    nc.tensor.matmul(
                                        pt[:], w_sb[nm][:, c, t, m * P:(m + 1) * P], rhs,
                                        start=first, stop=(c == NCHUNK - 1 and t == 8))
                                    first = False
                            nc.vector.bn_stats(out=st_t[m][:, j, :], in_=pt[:])
                    # per-channel mean/var over the 1024 spatial positions
                    mv = [stats_p.tile([P, 2], F32, name=f"mv_{nm}_{m}", tag="mv")
                          for m in range(2)]
                    for m in range(2):
                        nc.vector.bn_aggr(out=mv[m][:], in_=st_t[m][:])
                    prep = stats_p.tile([P, 4], F32, name=f"prep_{nm}", tag="prep")
                    for m in range(2):
                        nc.vector.tensor_copy(out=prep[:, 2 * m:2 * m + 1], in_=mv[m][:, 0:1])
                        sq = stats_p.tile([P, 1], F32, name=f"sq_{nm}_{m}", tag="sq")
                        nc.vector.tensor_mul(out=sq[:], in0=mv[m][:, 0:1], in1=mv[m][:, 0:1])
                        nc.vector.tensor_add(out=prep[:, 2 * m + 1:2 * m + 2],
                                             in0=mv[m][:, 1:2], in1=sq[:])
                    red = stats_p.tile([P, 4], F32, name=f"red_{nm}", tag="red")
                    nc.gpsimd.partition_all_reduce(red[:], prep[:], channels=P,
                                                   reduce_op=bass_isa.ReduceOp.add)
                    # mean = (c0+c2)/256 ; var = (c1+c3)/256 - mean^2
                    mt = stats_p.tile([P, 4], F32, name=f"mt_{nm}", tag="mt")
                    nc.vector.tensor_add(out=mt[:, 0:1], in0=red[:, 0:1], in1=red[:, 2:3])
                    nc.scalar.mul(out=mt[:, 0:1], in_=mt[:, 0:1], mul=1.0 / C)
                    nc.vector.tensor_add(out=mt[:, 1:2], in0=red[:, 1:2], in1=red[:, 3:4])
                    nc.scalar.mul(out=mt[:, 1:2], in_=mt[:, 1:2], mul=1.0 / C)
                    nc.vector.tensor_mul(out=mt[:, 2:3], in0=mt[:, 0:1], in1=mt[:, 0:1])
                    nc.vector.tensor_sub(out=mt[:, 1:2], in0=mt[:, 1:2], in1=mt[:, 2:3])
                    nc.scalar.activation(out=mt[:, 1:2], in_=mt[:, 1:2], func=AF.Sqrt,
                                         bias=eps_t[:], scale=1.0)
                    nc.vector.reciprocal(out=mt[:, 1:2], in_=mt[:, 1:2])
                    if DEBUG_TAPS:
                        nc.sync.dma_start(dbg["dbg_st"][:, iproj, :], mt[:])
                    gc = VGCOL[nm]
                    sc = stats_p.tile([P, 4], F32, name=f"sc_{nm}", tag="sc")
                    for m in range(2):
                        # s_m = g_m * rstd ; t_m = b_m - mean * s_m
                        nc.vector.tensor_mul(out=sc[:, m:m + 1],
                                             in0=vecs[:, gc + m:gc + m + 1], in1=mt[:, 1:2])
                        nc.vector.tensor_mul(out=sc[:, 2 + m:3 + m],
                                             in0=mt[:, 0:1], in1=sc[:, m:m + 1])
                        nc.vector.tensor_sub(out=sc[:, 2 + m:3 + m],
                                             in0=vecs[:, gc + 2 + m:gc + 3 + m],
                                             in1=sc[:, 2 + m:3 + m])
                    for m in range(2):
                        for j in range(2):
                            nc.scalar.activation(
                                out=dst[:, m, 512 * j:512 * (j + 1)], in_=ps_t[m * 2 + j][:],
                                func=AF.Gelu, scale=sc[:, m:m + 1], bias=sc[:, 2 + m:3 + m])

                # v transpose into v_aug via PE transpose (psum slots shared w/ conv tag)
                nc.vector.memset(v_aug[:, :, :, 32:33], 1.0)
                for k in range(NCHUNK):
                    for i in range(8):
                        pvt = psc.tile([P, P], BF16, name=f"vt_{k}_{i}", tag="conv")
                        nc.tensor.transpose(pvt[:], v_bf[:, k, P * i:P * (i + 1)], ident[:])
                        nc.scalar.copy(out=v_aug[:, i, 4 * k:4 * k + 4, 0:32], in_=pvt[:])

            if DEBUG_TAPS:
                nc.sync.dma_start(dbg["dbg_q"][:], q_sb[:])
                nc.sync.dma_start(dbg["dbg_k"][:], k_sb[:])
                nc.sync.dma_start(dbg["dbg_v"][:], v_bf[:])
                nc.sync.dma_start(dbg["dbg_va"][:], v_aug[:])

            # ---------------- attention ----------------
            from contextlib import ExitStack
            with ExitStack() as stk:
                attnT_p = stk.enter_context(tc.tile_pool(name="attnT_p", bufs=4))
                ebuf = stk.enter_context(tc.tile_pool(name="ebuf", bufs=4))
                ebias_p = stk.enter_context(tc.tile_pool(name="ebias_p", bufs=4))
                rc_p = stk.enter_context(tc.tile_pool(name="rc", bufs=2))
                ps_avp = stk.enter_context(tc.tile_pool(name="ps_avp", bufs=2, space="PSUM"))
                for pair in range(4):
                    grp = pair // 2
                    ps_av = ps_avp.tile([P, N], F32, name=f"av_{pair}", tag="av")
                    for h in (2 * pair, 2 * pair + 1):
                        g, r, rv = h // 4, h % 4, h % 2
                        attnT = attnT_p.tile([P, 8, N], BF16, name=f"attnT_{h}", tag="attnT")
                        for i in range(8):
                            ps_s = ps_sc.tile([P, N], F32, name=f"s_{h}_{i}", tag="sc")
                            for nj in range(2):
                                nc.tensor.matmul(
                                    ps_s[:, 512 * nj:512 * (nj + 1)],
                                    k_sb[32 * r:32 * r + 32, g, P * i:P * (i + 1)],
                                    q_sb[32 * r:32 * r + 32, g, 512 * nj:512 * (nj + 1)],
                                    start=True, stop=True, tile_position=(32 * r, 0))
                            e_bf = ebuf.tile([P, N], BF16, name=f"e_{h}_{i}", tag="e")
                            nc.scalar.activation(out=e_bf[:], in_=ps_s[:], func=AF.Exp,
                                                 scale=SCALE)
                            eb_t = ebias_p.tile([P, N], BF16, name=f"eb_{h}_{i}", tag="eb")
                            nc.sync.dma_start(eb_t[:], eb_d[h, i])
                            nc.vector.tensor_mul(out=attnT[:, i, :], in0=e_bf[:], in1=eb_t[:])
                        # attn @ [v | 1]: out rows 64*rv..+32, sums row 64*rv+32
                        for nj in range(2):
                            for i in range(8):
                                nc.tensor.matmul(
                                    ps_av[64 * rv:64 * rv + 33, 512 * nj:512 * (nj + 1)],
                                    v_aug[:, i, h, 0:33],
                                    attnT[:, i, 512 * nj:512 * (nj + 1)],
                                    start=(i == 0), stop=(i == 7), tile_position=(0, 64 * rv))
                    # normalize the pair: recip of sums via [128,16] respread
                    srows = rc_p.tile([64, N], F32, name=f"sr_{pair}", tag="sr")
                    sp = rc_p.tile([P, 16], F32, name=f"sp_{pair}", tag="sp")
                    rrow = rc_p.tile([64, N], F32, name=f"rr_{pair}", tag="rr")
                    rcp_bc = rc_p.tile([P, N], F32, name=f"rb_{pair}", tag="rb")
                    for rv in range(2):
                        nc.vector.tensor_copy(out=srows[32 * rv:32 * rv + 1, :],
                                              in_=ps_av[64 * rv + 32:64 * rv + 33, :])
                        nc.sync.dma_start(out=sp[:, 8 * rv:8 * rv + 8],
                                          in_=srows[32 * rv:32 * rv + 1, :])
                    nc.vector.reciprocal(out=sp[:], in_=sp[:])
                    for rv in range(2):
                        nc.sync.dma_start(out=rrow[32 * rv:32 * rv + 1, :],
                                          in_=sp[:, 8 * rv:8 * rv + 8])
                        rowap = rrow[32 * rv:32 * rv + 1, :]
                        src = bass.AP(tensor=rowap.tensor, offset=rowap.offset,
                                      ap=[list(rowap.ap[0]), [0, 32]]
                                      + [list(d) for d in rowap.ap[1:]])
                        nc.sync.dma_start(out=rcp_bc[64 * rv:64 * rv + 32, :], in_=src)
                    for h in (2 * pair, 2 * pair + 1):
                        r, rv = h % 4, h % 2
                        nc.vector.tensor_mul(out=attn_out[32 * r:32 * r + 32, grp, :],
                                             in0=ps_av[64 * rv:64 * rv + 32, :],
                                             in1=rcp_bc[64 * rv:64 * rv + 32, :])

                if DEBUG_TAPS:
                    nc.sync.dma_start(dbg["dbg_ao"][:], attn_out[:])

                # ---------------- 1x1 out conv ----------------
                for m in range(2):
                    for j in range(2):
                        ps_o = ps_sc.tile([P, 512], F32, name=f"o_{m}_{j}", tag="sc")
                        for k in range(NCHUNK):
                            nc.tensor.matmul(ps_o[:], wout_sb[:, k, m * P:(m + 1) * P],
                                             attn_out[:, k, 512 * j:512 * (j + 1)],
                                             start=(k == 0), stop=(k == NCHUNK - 1))
                        nc.scalar.activation(out=out_sb[:, m, 512 * j:512 * (j + 1)],
                                             in_=ps_o[:], func=AF.Identity,
                                             bias=vecs[:, 12 + m:13 + m], scale=1.0)
                nc.sync.dma_start(out_d[:], out_sb[:])

    nc.compile()
    return nc


def _prep_shared(Wq, gq, bq, Wk, gk, bk, Wv, gv, bv, bias_table, Wout, bout):
    def wt(W):
        # [co, ci, 3, 3] -> [ci%128, ci//128, tap, co]
        return np.ascontiguousarray(
            W.astype(np.float32).transpose(1, 2, 3, 0).reshape(NCHUNK, P, 9, C)
            .transpose(1, 0, 2, 3)).astype(ml_dtypes.bfloat16)
    vecs = np.zeros((P, 14), np.float32)
    for col, v in ((0, gq), (2, bq), (4, gk), (6, bk), (8, gv), (10, bv), (12, bout)):
        vecs[:, col] = v[:P]
        vecs[:, col + 1] = v[P:]
    wout = np.ascontiguousarray(Wout[:, :, 0, 0].T.reshape(NCHUNK, P, C)
                                .transpose(1, 0, 2)).astype(np.float32)
    idx = _rel_index()                     # [n, m]
    eb = np.exp(bias_table.astype(np.float32))[idx]   # [n, m, H]
    ebT = eb.transpose(2, 1, 0)            # [H, m, n]
    ebias = np.ascontiguousarray(ebT.reshape(HEADS, 8, P, N).astype(ml_dtypes.bfloat16))
    return {"wq": wt(Wq), "wk": wt(Wk), "wv": wt(Wv), "vecs": vecs,
            "wout": wout, "ebias": ebias}


def kernel(x, Wq, gq, bq, Wk, gk, bk, Wv, gv, bv, bias_table, Wout, bout):
    x = np.asarray(x, np.float32)
    if "nc" not in _cache:
        _cache["nc"] = build_nc()
    nc = _cache["nc"]
    shared = _prep_shared(np.asarray(Wq), np.asarray(gq), np.asarray(bq),
                          np.asarray(Wk), np.asarray(gk), np.asarray(bk),
                          np.asarray(Wv), np.asarray(gv), np.asarray(bv),
                          np.asarray(bias_table), np.asarray(Wout), np.asarray(bout))
    in_maps = []
    for b in range(B):
        m = dict(shared)
        m["x"] = np.ascontiguousarray(
            x[b].reshape(NCHUNK, P, IH, IW).astype(ml_dtypes.bfloat16))
        in_maps.append(m)
    _cache["last_in_maps"] = in_maps
    res = bass_utils.run_bass_kernel_spmd(nc, in_maps, core_ids=list(range(B)))
    out = np.stack([r["out"] for r in res.results])          # [B, 128, 2, 1024]
    out = out.transpose(0, 2, 1, 3).reshape(B, C, IH, IW)
    return np.ascontiguousarray(out.astype(np.float32))


if __name__ == "__main__":
    rng = np.random.default_rng(0)
    inputs = {
        'x': rng.standard_normal((B, C, IH, IW), dtype=np.float32),
        'Wq': (rng.standard_normal((C, C, 3, 3)) * 0.02).astype(np.float32),
        'gq': np.ones(C, np.float32), 'bq': np.zeros(C, np.float32),
        'Wk': (rng.standard_normal((C, C, 3, 3)) * 0.02).astype(np.float32),
        'gk': np.ones(C, np.float32), 'bk': np.zeros(C, np.float32),
        'Wv': (rng.standard_normal((C, C, 3, 3)) * 0.02).astype(np.float32),
        'gv': np.ones(C, np.float32), 'bv': np.zeros(C, np.float32),
        'bias_table': (rng.standard_normal(((2 * IH - 1) * (2 * IW - 1), HEADS)) * 0.02).astype(np.float32),
        'Wout': (rng.standard_normal((C, C, 1, 1)) * 0.02).astype(np.float32),
        'bout': np.zeros(C, np.float32),
    }
    out = kernel(**inputs)
    print("out", out.shape, out.dtype, np.abs(out).max())


# revision 26
# speedup vs baseline: 2.2183x; 1.0943x over previous
"""Trainium2 Bass kernel for nn_Attention_85169201480311.

Dense transformer block: 3x (conv3x3 -> GroupNorm(1) -> exact GELU) projections,
8-head attention over 1024 tokens with relative-position bias, 1x1 out-conv.

Sharding: data-parallel over batch (8 samples -> 8 cores), params replicated.

Per-core program:
 - conv3x3 = 18 PSUM-accumulating bf16 matmuls (2 cin chunks x 9 taps) against a
   zero-padded [128, 2, 34, 34] SBUF image.
 - GroupNorm stats: bn_stats/bn_aggr per psum tile + gpsimd partition_all_reduce;
   affine+GELU fused into the PSUM eviction via ACT scale/bias operands.
 - attention in scores^T layout (m on partitions): both matmuls consume the
   native conv output layout, K=32 scores matmuls row-packed 4-up via
   tile_position. Softmax without max-subtraction; relative-position bias
   applied multiplicatively post-exp: exp(s*S)*exp(bias), exp(bias) precomputed
   host-side (bf16) and streamed.
 - attn@v: lhsT = [v_head | ones] (M=33), so psum row 32/96 carries the softmax
   denominator; two heads per psum tile via tile_position col packing. The
   reciprocal runs on a [128,16] respread of the sums (DMA repartition), and the
   normalizing multiply reads psum directly, writing the partition-aligned
   [c, n] layout the 1x1 out-conv consumes.
"""
import sys
for p in ('/opt/trn_rl_repo', '/root/.axon_site/_ro/trn_rl_repo'):
    if p not in sys.path:
        sys.path.insert(0, p)

import numpy as np
import ml_dtypes

import concourse.bass as bass
import concourse.tile as tile
from concourse import mybir, bacc, bass_isa
from concourse import bass_utils
from concourse.masks import make_identity

F32 = mybir.dt.float32
BF16 = mybir.dt.bfloat16
AF = mybir.ActivationFunctionType

IH = IW = 32
N = IH * IW          # 1024 tokens
C = 256
HEADS = 8
DH = 32
SCALE = 32 ** -0.5
EPS = 1e-6
B = 8
P = 128
NCHUNK = C // P      # 2 channel chunks

_cache = {}
DEBUG_TAPS = False


def _rel_index():
    coords = np.stack(np.meshgrid(np.arange(IH), np.arange(IW), indexing='ij')).reshape(2, -1)
    rel = coords[:, :, None] - coords[:, None, :]
    rel[0] += IH - 1
    rel[1] += IW - 1
    rel[0] *= 2 * IW - 1
    return rel.sum(0)  # [n, m] int


def build_nc():
    nc = bacc.Bacc('TRN2', target_bir_lowering=False)

    x_d = nc.dram_tensor("x", [NCHUNK, P, IH, IW], BF16, kind="ExternalInput")
    w_d = {}
    for nm in ("wq", "wk", "wv"):
        w_d[nm] = nc.dram_tensor(nm, [P, NCHUNK, 9, C], BF16, kind="ExternalInput")
    wout_d = nc.dram_tensor("wout", [P, NCHUNK, C], F32, kind="ExternalInput")
    vecs_d = nc.dram_tensor("vecs", [P, 14], F32, kind="ExternalInput")
    eb_d = nc.dram_tensor("ebias", [HEADS, 8, P, N], BF16, kind="ExternalInput")
    out_d = nc.dram_tensor("out", [P, NCHUNK, N], F32, kind="ExternalOutput")
    dbg = {}
    if DEBUG_TAPS:
        for nm, shp, dt in (("dbg_q", [P, NCHUNK, N], BF16), ("dbg_k", [P, NCHUNK, N], BF16),
                            ("dbg_v", [P, NCHUNK, N], BF16), ("dbg_va", [P, 8, 8, 34], BF16),
                            ("dbg_st", [P, 3, 4], F32), ("dbg_ao", [P, NCHUNK, N], F32)):
            dbg[nm] = nc.dram_tensor(nm, shp, dt, kind="ExternalOutput")

    # vecs cols: gq0,gq1,bq0,bq1, gk0,gk1,bk0,bk1, gv0,gv1,bv0,bv1, bout0,bout1
    VGCOL = {"wq": 0, "wk": 4, "wv": 8}

    with tile.TileContext(nc) as tc:
        with tc.tile_pool(name="const", bufs=1) as const, \
             tc.tile_pool(name="proj", bufs=1) as proj, \
             tc.tile_pool(name="stats", bufs=2) as stats_p, \
             tc.tile_pool(name="attn", bufs=1) as attn_p:

            vecs = const.tile([P, 14], F32)
            nc.sync.dma_start(vecs[:], vecs_d[:])
            eps_t = const.tile([P, 1], F32)
            nc.vector.memset(eps_t[:], EPS)
            wout_sb = const.tile([P, NCHUNK, C], F32)
            nc.sync.dma_start(wout_sb[:], wout_d[:])
            ident = const.tile([P, P], BF16)
            make_identity(nc, ident[:])

            xpad = const.tile([P, NCHUNK, IH + 2, IW + 2], BF16)
            nc.gpsimd.memset(xpad[:], 0.0)
            for c in range(NCHUNK):
                nc.sync.dma_start(xpad[:, c, 1:IH + 1, 1:IW + 1], x_d[c])

            q_sb = proj.tile([P, NCHUNK, N], BF16)
            k_sb = proj.tile([P, NCHUNK, N], BF16)
            v_bf = proj.tile([P, NCHUNK, N], BF16)
            v_aug = proj.tile([P, 8, 8, 34], BF16)   # [*, i, h, 0:32 v | 32 ones | pad]
            attn_out = attn_p.tile([P, NCHUNK, N], F32)
            out_sb = attn_p.tile([P, NCHUNK, N], F32)

            # ---------------- projections ----------------
            with tc.tile_pool(name="wpool", bufs=1) as wpool, \
                 tc.tile_pool(name="psc", bufs=6, space="PSUM") as psc:
                w_sb = {}
                for nm in ("wq", "wk", "wv"):
                    w_sb[nm] = wpool.tile([P, NCHUNK, 9, C], BF16, name=f"sb_{nm}")
                    nc.sync.dma_start(w_sb[nm][:], w_d[nm][:])

                for iproj, (nm, dst) in enumerate(
                        (("wq", q_sb), ("wk", k_sb), ("wv", v_bf))):
                    ps_t = [psc.tile([P, 512], F32, name=f"cv_{nm}_{m}_{j}", tag="conv")
                            for m in range(2) for j in range(2)]
                    st_t = [stats_p.tile([P, 2, 6], F32, name=f"st_{nm}_{m}", tag="stats")
                            for m in range(2)]
                    for m in range(2):
                        for j in range(2):
                            pt = ps_t[m * 2 + j]
                            first = True
                            for c in range(NCHUNK):
                                for t in range(9):
                                    dy, dx = t // 3, t % 3
                                    rhs = xpad[:, c, 16 * j + dy:16 * j + dy + 16, dx:dx + 32]
                                    nc.tensor.matmul(
                                        pt[:], w_sb[nm][:, c, t, m * P:(m + 1) * P], rhs,
                                        start=first, stop=(c == NCHUNK - 1 and t == 8))
                                    first = False
                            nc.vector.bn_stats(out=st_t[m][:, j, :], in_=pt[:])
                    # per-channel mean/var over the 1024 spatial positions
                    mv = [stats_p.tile([P, 2], F32, name=f"mv_{nm}_{m}", tag="mv")
                          for m in range(2)]
                    for m in range(2):
                        nc.vector.bn_aggr(out=mv[m][:], in_=st_t[m][:])
                    prep = stats_p.tile([P, 4], F32, name=f"prep_{nm}", tag="prep")
                    for m in range(2):
                        nc.vector.tensor_copy(out=prep[:, 2 * m:2 * m + 1], in_=mv[m][:, 0:1])
                        sq = stats_p.tile([P, 1], F32, name=f"sq_{nm}_{m}", tag="sq")
                        nc.vector.tensor_mul(out=sq[:], in0=mv[m][:, 0:1], in1=mv[m][:, 0:1])
                        nc.vector.tensor_add(out=prep[:, 2 * m + 1:2 * m + 2],
                                             in0=mv[m][:, 1:2], in1=sq[:])
                    red = stats_p.tile([P, 4], F32, name=f"red_{nm}", tag="red")
                    nc.gpsimd.partition_all_reduce(red[:], prep[:], channels=P,
                                                   reduce_op=bass_isa.ReduceOp.add)
                    # mean = (c0+c2)/256 ; var = (c1+c3)/256 - mean^2
                    mt = stats_p.tile([P, 4], F32, name=f"mt_{nm}", tag="mt")
                    nc.vector.tensor_add(out=mt[:, 0:1], in0=red[:, 0:1], in1=red[:, 2:3])
                    nc.scalar.mul(out=mt[:, 0:1], in_=mt[:, 0:1], mul=1.0 / C)
                    nc.vector.tensor_add(out=mt[:, 1:2], in0=red[:, 1:2], in1=red[:, 3:4])
                    nc.scalar.mul(out=mt[:, 1:2], in_=mt[:, 1:2], mul=1.0 / C)
                    nc.vector.tensor_mul(out=mt[:, 2:3], in0=mt[:, 0:1], in1=mt[:, 0:1])
                    nc.vector.tensor_sub(out=mt[:, 1:2], in0=mt[:, 1:2], in1=mt[:, 2:3])
                    nc.scalar.activation(out=mt[:, 1:2], in_=mt[:, 1:2], func=AF.Sqrt,
                                         bias=eps_t[:], scale=1.0)
                    nc.vector.reciprocal(out=mt[:, 1:2], in_=mt[:, 1:2])
                    if DEBUG_TAPS:
                        nc.sync.dma_start(dbg["dbg_st"][:, iproj, :], mt[:])
                    gc = VGCOL[nm]
                    sc = stats_p.tile([P, 4], F32, name=f"sc_{nm}", tag="sc")
                    for m in range(2):
                        # s_m = g_m * rstd ; t_m = b_m - mean * s_m
                        nc.vector.tensor_mul(out=sc[:, m:m + 1],
                                             in0=vecs[:, gc + m:gc + m + 1], in1=mt[:, 1:2])
                        nc.vector.tensor_mul(out=sc[:, 2 + m:3 + m],
                                             in0=mt[:, 0:1], in1=sc[:, m:m + 1])
                        nc.vector.tensor_sub(out=sc[:, 2 + m:3 + m],
                                             in0=vecs[:, gc + 2 + m:gc + 3 + m],
                                             in1=sc[:, 2 + m:3 + m])
                    for m in range(2):
                        for j in range(2):
                            nc.scalar.activation(
                                out=dst[:, m, 512 * j:512 * (j + 1)], in_=ps_t[m * 2 + j][:],
                                func=AF.Gelu, scale=sc[:, m:m + 1], bias=sc[:, 2 + m:3 + m])

                # v transpose into v_aug via PE transpose (psum slots shared w/ conv tag)
                nc.vector.memset(v_aug[:, :, :, 32:33], 1.0)
                for k in range(NCHUNK):
                    for i in range(8):
                        pvt = psc.tile([P, P], BF16, name=f"vt_{k}_{i}", tag="conv")
                        nc.tensor.transpose(pvt[:], v_bf[:, k, P * i:P * (i + 1)], ident[:])
                        nc.scalar.copy(out=v_aug[:, i, 4 * k:4 * k + 4, 0:32], in_=pvt[:])

            if DEBUG_TAPS:
                nc.sync.dma_start(dbg["dbg_q"][:], q_sb[:])
                nc.sync.dma_start(dbg["dbg_k"][:], k_sb[:])
                nc.sync.dma_start(dbg["dbg_v"][:], v_bf[:])
                nc.sync.dma_start(dbg["dbg_va"][:], v_aug[:])

            # ---------------- attention ----------------
            from contextlib import ExitStack
            with ExitStack() as stk:
                attnT_p = stk.enter_context(tc.tile_pool(name="attnT_p", bufs=4))
                ebuf = stk.enter_context(tc.tile_pool(name="ebuf", bufs=4))
                ebias_p = stk.enter_context(tc.tile_pool(name="ebias_p", bufs=4))
                rc_p = stk.enter_context(tc.tile_pool(name="rc", bufs=2))
                ps_sc = stk.enter_context(tc.tile_pool(name="ps_sc", bufs=2, space="PSUM"))
                ps_avp = stk.enter_context(tc.tile_pool(name="ps_avp", bufs=2, space="PSUM"))
                for pair in range(4):
                    grp = pair // 2
                    h0, h1 = 2 * pair, 2 * pair + 1
                    ps_av = ps_avp.tile([P, N], F32, name=f"av_{pair}", tag="av")
                    attnT = {h: attnT_p.tile([P, 8, N], BF16, name=f"attnT_{h}", tag="attnT")
                             for h in (h0, h1)}
                    # interleave the two heads so their matmuls run concurrently
                    # in distinct PE row groups
                    for i in range(8):
                        ps_s = {}
                        for h in (h0, h1):
                            g, r = h // 4, h % 4
                            ps_s[h] = ps_sc.tile([P, N], F32, name=f"s_{h}_{i}", tag="sc")
                            for nj in range(2):
                                nc.tensor.matmul(
                                    ps_s[h][:, 512 * nj:512 * (nj + 1)],
                                    k_sb[32 * r:32 * r + 32, g, P * i:P * (i + 1)],
                                    q_sb[32 * r:32 * r + 32, g, 512 * nj:512 * (nj + 1)],
                                    start=True, stop=True, tile_position=(32 * r, 0))
                        for h in (h0, h1):
                            e_bf = ebuf.tile([P, N], BF16, name=f"e_{h}_{i}", tag="e")
                            nc.scalar.activation(out=e_bf[:], in_=ps_s[h][:], func=AF.Exp,
                                                 scale=SCALE)
                            eb_t = ebias_p.tile([P, N], BF16, name=f"eb_{h}_{i}", tag="eb")
                            nc.sync.dma_start(eb_t[:], eb_d[h, i])
                            nc.vector.tensor_mul(out=attnT[h][:, i, :], in0=e_bf[:],
                                                 in1=eb_t[:])
                    # attn @ [v | 1]: head h at col group 64*(h%2); psum row
                    # 64*(h%2)+32 carries the softmax denominator
                    for nj in range(2):
                        for i in range(8):
                            for h in (h0, h1):
                                rv = h % 2
                                nc.tensor.matmul(
                                    ps_av[64 * rv:64 * rv + 33, 512 * nj:512 * (nj + 1)],
                                    v_aug[:, i, h, 0:33],
                                    attnT[h][:, i, 512 * nj:512 * (nj + 1)],
                                    start=(i == 0), stop=(i == 7), tile_position=(0, 64 * rv))
                    # normalize the pair: recip of sums via [128,16] respread
                    srows = rc_p.tile([64, N], F32, name=f"sr_{pair}", tag="sr")
                    sp = rc_p.tile([P, 16], F32, name=f"sp_{pair}", tag="sp")
                    rrow = rc_p.tile([64, N], F32, name=f"rr_{pair}", tag="rr")
                    rcp_bc = rc_p.tile([P, N], F32, name=f"rb_{pair}", tag="rb")
                    for rv in range(2):
                        nc.vector.tensor_copy(out=srows[32 * rv:32 * rv + 1, :],
                                              in_=ps_av[64 * rv + 32:64 * rv + 33, :])
                        nc.sync.dma_start(out=sp[:, 8 * rv:8 * rv + 8],
                                          in_=srows[32 * rv:32 * rv + 1, :])
                    nc.vector.reciprocal(out=sp[:], in_=sp[:])
                    for rv in range(2):
                        nc.sync.dma_start(out=rrow[32 * rv:32 * rv + 1, :],
                                          in_=sp[:, 8 * rv:8 * rv + 8])
                        rowap = rrow[32 * rv:32 * rv + 1, :]
                        src = bass.AP(tensor=rowap.tensor, offset=rowap.offset,
                                      ap=[list(rowap.ap[0]), [0, 32]]
                                      + [list(d) for d in rowap.ap[1:]])
                        nc.sync.dma_start(out=rcp_bc[64 * rv:64 * rv + 32, :], in_=src)
                    for h in (2 * pair, 2 * pair + 1):
                        r, rv = h % 4, h % 2
                        nc.vector.tensor_mul(out=attn_out[32 * r:32 * r + 32, grp, :],
                                             in0=ps_av[64 * rv:64 * rv + 32, :],
                                             in1=rcp_bc[64 * rv:64 * rv + 32, :])

                if DEBUG_TAPS:
                    nc.sync.dma_start(dbg["dbg_ao"][:], attn_out[:])

                # ---------------- 1x1 out conv ----------------
                for m in range(2):
                    for j in range(2):
                        ps_o = ps_sc.tile([P, 512], F32, name=f"o_{m}_{j}", tag="sc")
                        for k in range(NCHUNK):
                            nc.tensor.matmul(ps_o[:], wout_sb[:, k, m * P:(m + 1) * P],
                                             attn_out[:, k, 512 * j:512 * (j + 1)],
                                             start=(k == 0), stop=(k == NCHUNK - 1))
                        nc.scalar.activation(out=out_sb[:, m, 512 * j:512 * (j + 1)],
                                             in_=ps_o[:], func=AF.Identity,
                                             bias=vecs[:, 12 + m:13 + m], scale=1.0)
                nc.sync.dma_start(out_d[:], out_sb[:])

    nc.compile()
    return nc


def _prep_shared(Wq, gq, bq, Wk, gk, bk, Wv, gv, bv, bias_table, Wout, bout):
    def wt(W):
        # [co, ci, 3, 3] -> [ci%128, ci//128, tap, co]
        return np.ascontiguousarray(
            W.astype(np.float32).transpose(1, 2, 3, 0).reshape(NCHUNK, P, 9, C)
            .transpose(1, 0, 2, 3)).astype(ml_dtypes.bfloat16)
    vecs = np.zeros((P, 14), np.float32)
    for col, v in ((0, gq), (2, bq), (4, gk), (6, bk), (8, gv), (10, bv), (12, bout)):
        vecs[:, col] = v[:P]
        vecs[:, col + 1] = v[P:]
    wout = np.ascontiguousarray(Wout[:, :, 0, 0].T.reshape(NCHUNK, P, C)
                                .transpose(1, 0, 2)).astype(np.float32)
    idx = _rel_index()                     # [n, m]
    eb = np.exp(bias_table.astype(np.float32))[idx]   # [n, m, H]
    ebT = eb.transpose(2, 1, 0)            # [H, m, n]
    ebias = np.ascontiguousarray(ebT.reshape(HEADS, 8, P, N).astype(ml_dtypes.bfloat16))
    return {"wq": wt(Wq), "wk": wt(Wk), "wv": wt(Wv), "vecs": vecs,
            "wout": wout, "ebias": ebias}


def kernel(x, Wq, gq, bq, Wk, gk, bk, Wv, gv, bv, bias_table, Wout, bout):
    x = np.asarray(x, np.float32)
    if "nc" not in _cache:
        _cache["nc"] = build_nc()
    nc = _cache["nc"]
    shared = _prep_shared(np.asarray(Wq), np.asarray(gq), np.asarray(bq),
                          np.asarray(Wk), np.asarray(gk), np.asarray(bk),
                          np.asarray(Wv), np.asarray(gv), np.asarray(bv),
                          np.asarray(bias_table), np.asarray(Wout), np.asarray(bout))
    in_maps = []
    for b in range(B):
        m = dict(shared)
        m["x"] = np.ascontiguousarray(
            x[b].reshape(NCHUNK, P, IH, IW).astype(ml_dtypes.bfloat16))
        in_maps.append(m)
    _cache["last_in_maps"] = in_maps
    res = bass_utils.run_bass_kernel_spmd(nc, in_maps, core_ids=list(range(B)))
    out = np.stack([r["out"] for r in res.results])          # [B, 128, 2, 1024]
    out = out.transpose(0, 2, 1, 3).reshape(B, C, IH, IW)
    return np.ascontiguousarray(out.astype(np.float32))


if __name__ == "__main__":
    rng = np.random.default_rng(0)
    inputs = {
        'x': rng.standard_normal((B, C, IH, IW), dtype=np.float32),
        'Wq': (rng.standard_normal((C, C, 3, 3)) * 0.02).astype(np.float32),
        'gq': np.ones(C, np.float32), 'bq': np.zeros(C, np.float32),
        'Wk': (rng.standard_normal((C, C, 3, 3)) * 0.02).astype(np.float32),
        'gk': np.ones(C, np.float32), 'bk': np.zeros(C, np.float32),
        'Wv': (rng.standard_normal((C, C, 3, 3)) * 0.02).astype(np.float32),
        'gv': np.ones(C, np.float32), 'bv': np.zeros(C, np.float32),
        'bias_table': (rng.standard_normal(((2 * IH - 1) * (2 * IW - 1), HEADS)) * 0.02).astype(np.float32),
        'Wout': (rng.standard_normal((C, C, 1, 1)) * 0.02).astype(np.float32),
        'bout': np.zeros(C, np.float32),
    }
    out = kernel(**inputs)
    print("out", out.shape, out.dtype, np.abs(out).max())


# revision 30
# speedup vs baseline: 2.2457x; 1.0123x over previous
"""Trainium2 Bass kernel for nn_Attention_85169201480311.

Dense transformer block: 3x (conv3x3 -> GroupNorm(1) -> exact GELU) projections,
8-head attention over 1024 tokens with relative-position bias, 1x1 out-conv.

Sharding: data-parallel over batch (8 samples -> 8 cores), params replicated.

Per-core program:
 - conv3x3 = 18 PSUM-accumulating bf16 matmuls (2 cin chunks x 9 taps) against a
   zero-padded [128, 2, 34, 34] SBUF image.
 - GroupNorm stats: bn_stats/bn_aggr per psum tile + gpsimd partition_all_reduce;
   affine+GELU fused into the PSUM eviction via ACT scale/bias operands.
 - attention in scores^T layout (m on partitions): both matmuls consume the
   native conv output layout, K=32 scores matmuls row-packed 4-up via
   tile_position. Softmax without max-subtraction; relative-position bias
   applied multiplicatively post-exp: exp(s*S)*exp(bias), exp(bias) precomputed
   host-side (bf16) and streamed.
 - attn@v: lhsT = [v_head | ones] (M=33), so psum row 32/96 carries the softmax
   denominator; two heads per psum tile via tile_position col packing. The
   reciprocal runs on a [128,16] respread of the sums (DMA repartition), and the
   normalizing multiply reads psum directly, writing the partition-aligned
   [c, n] layout the 1x1 out-conv consumes.
"""
import sys
for p in ('/opt/trn_rl_repo', '/root/.axon_site/_ro/trn_rl_repo'):
    if p not in sys.path:
        sys.path.insert(0, p)

import numpy as np
import ml_dtypes

import concourse.bass as bass
import concourse.tile as tile
from concourse import mybir, bacc, bass_isa
from concourse import bass_utils
from concourse.masks import make_identity

F32 = mybir.dt.float32
BF16 = mybir.dt.bfloat16
AF = mybir.ActivationFunctionType

IH = IW = 32
N = IH * IW          # 1024 tokens
C = 256
HEADS = 8
DH = 32
SCALE = 32 ** -0.5
EPS = 1e-6
B = 8
P = 128
NCHUNK = C // P      # 2 channel chunks

_cache = {}
DEBUG_TAPS = False


def _rel_index():
    coords = np.stack(np.meshgrid(np.arange(IH), np.arange(IW), indexing='ij')).reshape(2, -1)
    rel = coords[:, :, None] - coords[:, None, :]
    rel[0] += IH - 1
    rel[1] += IW - 1
    rel[0] *= 2 * IW - 1
    return rel.sum(0)  # [n, m] int


def build_nc():
    nc = bacc.Bacc('TRN2', target_bir_lowering=False)

    x_d = nc.dram_tensor("x", [NCHUNK, P, IH, IW], BF16, kind="ExternalInput")
    w_d = {}
    for nm in ("wq", "wk", "wv"):
        w_d[nm] = nc.dram_tensor(nm, [P, NCHUNK, 9, C], BF16, kind="ExternalInput")
    wout_d = nc.dram_tensor("wout", [P, NCHUNK, C], F32, kind="ExternalInput")
    vecs_d = nc.dram_tensor("vecs", [P, 14], F32, kind="ExternalInput")
    eb_d = nc.dram_tensor("ebias", [HEADS, 8, P, N], BF16, kind="ExternalInput")
    out_d = nc.dram_tensor("out", [P, NCHUNK, N], F32, kind="ExternalOutput")
    dbg = {}
    if DEBUG_TAPS:
        for nm, shp, dt in (("dbg_q", [P, NCHUNK, N], BF16), ("dbg_k", [P, NCHUNK, N], BF16),
                            ("dbg_v", [P, NCHUNK, N], BF16), ("dbg_va", [P, 8, 8, 34], BF16),
                            ("dbg_st", [P, 3, 4], F32), ("dbg_ao", [P, NCHUNK, N], F32)):
            dbg[nm] = nc.dram_tensor(nm, shp, dt, kind="ExternalOutput")

    # vecs cols: gq0,gq1,bq0,bq1, gk0,gk1,bk0,bk1, gv0,gv1,bv0,bv1, bout0,bout1
    VGCOL = {"wq": 0, "wk": 4, "wv": 8}

    with tile.TileContext(nc) as tc:
        with tc.tile_pool(name="const", bufs=1) as const, \
             tc.tile_pool(name="proj", bufs=1) as proj, \
             tc.tile_pool(name="stats", bufs=2) as stats_p, \
             tc.tile_pool(name="attn", bufs=1) as attn_p:

            xpad = const.tile([P, NCHUNK, IH + 2, IW + 2], BF16)
            nc.vector.memset(xpad[:], 0.0)
            for c in range(NCHUNK):
                nc.sync.dma_start(xpad[:, c, 1:IH + 1, 1:IW + 1], x_d[c])
            vecs = const.tile([P, 14], F32)
            nc.sync.dma_start(vecs[:], vecs_d[:])
            eps_t = const.tile([P, 1], F32)
            nc.vector.memset(eps_t[:], EPS)
            wout_sb = const.tile([P, NCHUNK, C], F32)
            nc.sync.dma_start(wout_sb[:], wout_d[:])
            ident = const.tile([P, P], BF16)
            make_identity(nc, ident[:])

            q_sb = proj.tile([P, NCHUNK, N], BF16)
            k_sb = proj.tile([P, NCHUNK, N], BF16)
            v_bf = proj.tile([P, NCHUNK, N], BF16)
            v_aug = proj.tile([P, 8, 8, 34], BF16)   # [*, i, h, 0:32 v | 32 ones | pad]
            attn_out = attn_p.tile([P, NCHUNK, N], F32)
            out_sb = attn_p.tile([P, NCHUNK, N], F32)

            # ---------------- projections ----------------
            with tc.tile_pool(name="wpool", bufs=1) as wpool, \
                 tc.tile_pool(name="psc", bufs=6, space="PSUM") as psc:
                # warm up the PE HAM clock gate while the weight DMAs stream in
                warm_sb = wpool.tile([P, 512], BF16)
                nc.vector.memset(warm_sb[:], 0.0)
                ps_w = psc.tile([P, 512], F32, name="warm_ps", tag="warm", bufs=1)
                for _ in range(30):
                    nc.tensor.matmul(ps_w[:], warm_sb[:, 0:P], warm_sb[:],
                                     start=True, stop=True)
                w_sb = {}
                for nm in ("wq", "wk", "wv"):
                    w_sb[nm] = wpool.tile([P, NCHUNK, 9, C], BF16, name=f"sb_{nm}")
                    nc.sync.dma_start(w_sb[nm][:], w_d[nm][:])

                for iproj, (nm, dst) in enumerate(
                        (("wq", q_sb), ("wk", k_sb), ("wv", v_bf))):
                    ps_t = [psc.tile([P, 512], F32, name=f"cv_{nm}_{m}_{j}", tag="conv")
                            for m in range(2) for j in range(2)]
                    st_t = [stats_p.tile([P, 2, 6], F32, name=f"st_{nm}_{m}", tag="stats")
                            for m in range(2)]
                    for m in range(2):
                        for j in range(2):
                            pt = ps_t[m * 2 + j]
                            first = True
                            for c in range(NCHUNK):
                                for t in range(9):
                                    dy, dx = t // 3, t % 3
                                    rhs = xpad[:, c, 16 * j + dy:16 * j + dy + 16, dx:dx + 32]
                                    nc.tensor.matmul(
                                        pt[:], w_sb[nm][:, c, t, m * P:(m + 1) * P], rhs,
                                        start=first, stop=(c == NCHUNK - 1 and t == 8))
                                    first = False
                            nc.vector.bn_stats(out=st_t[m][:, j, :], in_=pt[:])
                    # per-channel mean/var over the 1024 spatial positions
                    mv = [stats_p.tile([P, 2], F32, name=f"mv_{nm}_{m}", tag="mv")
                          for m in range(2)]
                    for m in range(2):
                        nc.vector.bn_aggr(out=mv[m][:], in_=st_t[m][:])
                    prep = stats_p.tile([P, 4], F32, name=f"prep_{nm}", tag="prep")
                    for m in range(2):
                        nc.vector.tensor_copy(out=prep[:, 2 * m:2 * m + 1], in_=mv[m][:, 0:1])
                        sq = stats_p.tile([P, 1], F32, name=f"sq_{nm}_{m}", tag="sq")
                        nc.vector.tensor_mul(out=sq[:], in0=mv[m][:, 0:1], in1=mv[m][:, 0:1])
                        nc.vector.tensor_add(out=prep[:, 2 * m + 1:2 * m + 2],
                                             in0=mv[m][:, 1:2], in1=sq[:])
                    red = stats_p.tile([P, 4], F32, name=f"red_{nm}", tag="red")
                    nc.gpsimd.partition_all_reduce(red[:], prep[:], channels=P,
                                                   reduce_op=bass_isa.ReduceOp.add)
                    # mean = (c0+c2)/256 ; var = (c1+c3)/256 - mean^2
                    mt = stats_p.tile([P, 4], F32, name=f"mt_{nm}", tag="mt")
                    nc.vector.tensor_add(out=mt[:, 0:1], in0=red[:, 0:1], in1=red[:, 2:3])
                    nc.scalar.mul(out=mt[:, 0:1], in_=mt[:, 0:1], mul=1.0 / C)
                    nc.vector.tensor_add(out=mt[:, 1:2], in0=red[:, 1:2], in1=red[:, 3:4])
                    nc.scalar.mul(out=mt[:, 1:2], in_=mt[:, 1:2], mul=1.0 / C)
                    nc.vector.tensor_mul(out=mt[:, 2:3], in0=mt[:, 0:1], in1=mt[:, 0:1])
                    nc.vector.tensor_sub(out=mt[:, 1:2], in0=mt[:, 1:2], in1=mt[:, 2:3])
                    nc.scalar.activation(out=mt[:, 1:2], in_=mt[:, 1:2], func=AF.Sqrt,
                                         bias=eps_t[:], scale=1.0)
                    nc.vector.reciprocal(out=mt[:, 1:2], in_=mt[:, 1:2])
                    if DEBUG_TAPS:
                        nc.sync.dma_start(dbg["dbg_st"][:, iproj, :], mt[:])
                    gc = VGCOL[nm]
                    sc = stats_p.tile([P, 4], F32, name=f"sc_{nm}", tag="sc")
                    for m in range(2):
                        # s_m = g_m * rstd ; t_m = b_m - mean * s_m
                        nc.vector.tensor_mul(out=sc[:, m:m + 1],
                                             in0=vecs[:, gc + m:gc + m + 1], in1=mt[:, 1:2])
                        nc.vector.tensor_mul(out=sc[:, 2 + m:3 + m],
                                             in0=mt[:, 0:1], in1=sc[:, m:m + 1])
                        nc.vector.tensor_sub(out=sc[:, 2 + m:3 + m],
                                             in0=vecs[:, gc + 2 + m:gc + 3 + m],
                                             in1=sc[:, 2 + m:3 + m])
                    for m in range(2):
                        for j in range(2):
                            nc.scalar.activation(
                                out=dst[:, m, 512 * j:512 * (j + 1)], in_=ps_t[m * 2 + j][:],
                                func=AF.Gelu, scale=sc[:, m:m + 1], bias=sc[:, 2 + m:3 + m])

                # v transpose into v_aug via PE transpose (psum slots shared w/ conv tag)
                nc.vector.memset(v_aug[:, :, :, 32:33], 1.0)
                for k in range(NCHUNK):
                    for i in range(8):
                        pvt = psc.tile([P, P], BF16, name=f"vt_{k}_{i}", tag="conv")
                        nc.tensor.transpose(pvt[:], v_bf[:, k, P * i:P * (i + 1)], ident[:])
                        nc.scalar.copy(out=v_aug[:, i, 4 * k:4 * k + 4, 0:32], in_=pvt[:])

            if DEBUG_TAPS:
                nc.sync.dma_start(dbg["dbg_q"][:], q_sb[:])
                nc.sync.dma_start(dbg["dbg_k"][:], k_sb[:])
                nc.sync.dma_start(dbg["dbg_v"][:], v_bf[:])
                nc.sync.dma_start(dbg["dbg_va"][:], v_aug[:])

            # ---------------- attention ----------------
            from contextlib import ExitStack
            with ExitStack() as stk:
                attnT_p = stk.enter_context(tc.tile_pool(name="attnT_p", bufs=4))
                ebuf = stk.enter_context(tc.tile_pool(name="ebuf", bufs=6))
                ebias_p = stk.enter_context(tc.tile_pool(name="ebias_p", bufs=6))
                rc_p = stk.enter_context(tc.tile_pool(name="rc", bufs=2))
                ps_sc = stk.enter_context(tc.tile_pool(name="ps_sc", bufs=2, space="PSUM"))
                ps_avp = stk.enter_context(tc.tile_pool(name="ps_avp", bufs=2, space="PSUM"))
                attnTs = {}

                def scores_chain(pair):
                    h0, h1 = 2 * pair, 2 * pair + 1
                    for h in (h0, h1):
                        attnTs[h] = attnT_p.tile([P, 8, N], BF16, name=f"attnT_{h}",
                                                 tag="attnT")
                    # interleave the two heads so their matmuls run concurrently
                    # in distinct PE row groups
                    for i in range(8):
                        ps_s = {}
                        for h in (h0, h1):
                            g, r = h // 4, h % 4
                            ps_s[h] = ps_sc.tile([P, N], F32, name=f"s_{h}_{i}", tag="sc")
                            for nj in range(2):
                                nc.tensor.matmul(
                                    ps_s[h][:, 512 * nj:512 * (nj + 1)],
                                    k_sb[32 * r:32 * r + 32, g, P * i:P * (i + 1)],
                                    q_sb[32 * r:32 * r + 32, g, 512 * nj:512 * (nj + 1)],
                                    start=True, stop=True, tile_position=(32 * r, 0))
                        for h in (h0, h1):
                            e_bf = ebuf.tile([P, N], BF16, name=f"e_{h}_{i}", tag="e")
                            nc.scalar.activation(out=e_bf[:], in_=ps_s[h][:], func=AF.Exp,
                                                 scale=SCALE)
                            eb_t = ebias_p.tile([P, N], BF16, name=f"eb_{h}_{i}", tag="eb")
                            nc.sync.dma_start(eb_t[:], eb_d[h, i])
                            nc.vector.tensor_mul(out=attnTs[h][:, i, :], in0=e_bf[:],
                                                 in1=eb_t[:])

                def av_norm(pair):
                    grp = pair // 2
                    h0, h1 = 2 * pair, 2 * pair + 1
                    ps_av = ps_avp.tile([P, N], F32, name=f"av_{pair}", tag="av")
                    # attn @ [v | 1]: head h at col group 64*(h%2); psum row
                    # 64*(h%2)+32 carries the softmax denominator
                    for nj in range(2):
                        for i in range(8):
                            for h in (h0, h1):
                                rv = h % 2
                                nc.tensor.matmul(
                                    ps_av[64 * rv:64 * rv + 33, 512 * nj:512 * (nj + 1)],
                                    v_aug[:, i, h, 0:33],
                                    attnTs[h][:, i, 512 * nj:512 * (nj + 1)],
                                    start=(i == 0), stop=(i == 7),
                                    tile_position=(0, 64 * rv))
                    # normalize the pair: recip of sums via [128,16] respread
                    srows = rc_p.tile([64, N], F32, name=f"sr_{pair}", tag="sr")
                    sp = rc_p.tile([P, 16], F32, name=f"sp_{pair}", tag="sp")
                    rrow = rc_p.tile([64, N], F32, name=f"rr_{pair}", tag="rr")
                    rcp_bc = rc_p.tile([P, N], F32, name=f"rb_{pair}", tag="rb")
                    for rv in range(2):
                        nc.vector.tensor_copy(out=srows[32 * rv:32 * rv + 1, :],
                                              in_=ps_av[64 * rv + 32:64 * rv + 33, :])
                        nc.sync.dma_start(out=sp[:, 8 * rv:8 * rv + 8],
                                          in_=srows[32 * rv:32 * rv + 1, :])
                    nc.vector.reciprocal(out=sp[:], in_=sp[:])
                    for rv in range(2):
                        nc.sync.dma_start(out=rrow[32 * rv:32 * rv + 1, :],
                                          in_=sp[:, 8 * rv:8 * rv + 8])
                        rowap = rrow[32 * rv:32 * rv + 1, :]
                        src = bass.AP(tensor=rowap.tensor, offset=rowap.offset,
                                      ap=[list(rowap.ap[0]), [0, 32]]
                                      + [list(d) for d in rowap.ap[1:]])
                        nc.sync.dma_start(out=rcp_bc[64 * rv:64 * rv + 32, :], in_=src)
                    for h in (h0, h1):
                        r, rv = h % 4, h % 2
                        nc.vector.tensor_mul(out=attn_out[32 * r:32 * r + 32, grp, :],
                                             in0=ps_av[64 * rv:64 * rv + 32, :],
                                             in1=rcp_bc[64 * rv:64 * rv + 32, :])

                # software pipeline: av/normalize of pair p-1 is emitted after the
                # scores/exp chain of pair p so the PE never head-of-line blocks
                scores_chain(0)
                for pair in range(1, 4):
                    scores_chain(pair)
                    av_norm(pair - 1)
                av_norm(3)

                if DEBUG_TAPS:
                    nc.sync.dma_start(dbg["dbg_ao"][:], attn_out[:])

                # ---------------- 1x1 out conv ----------------
                for m in range(2):
                    for j in range(2):
                        ps_o = ps_sc.tile([P, 512], F32, name=f"o_{m}_{j}", tag="sc")
                        for k in range(NCHUNK):
                            nc.tensor.matmul(ps_o[:], wout_sb[:, k, m * P:(m + 1) * P],
                                             attn_out[:, k, 512 * j:512 * (j + 1)],
                                             start=(k == 0), stop=(k == NCHUNK - 1))
                        nc.scalar.activation(out=out_sb[:, m, 512 * j:512 * (j + 1)],
                                             in_=ps_o[:], func=AF.Identity,
                                             bias=vecs[:, 12 + m:13 + m], scale=1.0)
                nc.sync.dma_start(out_d[:], out_sb[:])

    nc.compile()
    return nc


def _prep_shared(Wq, gq, bq, Wk, gk, bk, Wv, gv, bv, bias_table, Wout, bout):
    def wt(W):
        # [co, ci, 3, 3] -> [ci%128, ci//128, tap, co]
        return np.ascontiguousarray(
            W.astype(np.float32).transpose(1, 2, 3, 0).reshape(NCHUNK, P, 9, C)
            .transpose(1, 0, 2, 3)).astype(ml_dtypes.bfloat16)
    vecs = np.zeros((P, 14), np.float32)
    for col, v in ((0, gq), (2, bq), (4, gk), (6, bk), (8, gv), (10, bv), (12, bout)):
        vecs[:, col] = v[:P]
        vecs[:, col + 1] = v[P:]
    wout = np.ascontiguousarray(Wout[:, :, 0, 0].T.reshape(NCHUNK, P, C)
                                .transpose(1, 0, 2)).astype(np.float32)
    idx = _rel_index()                     # [n, m]
    eb = np.exp(bias_table.astype(np.float32))[idx]   # [n, m, H]
    ebT = eb.transpose(2, 1, 0)            # [H, m, n]
    ebias = np.ascontiguousarray(ebT.reshape(HEADS, 8, P, N).astype(ml_dtypes.bfloat16))
    return {"wq": wt(Wq), "wk": wt(Wk), "wv": wt(Wv), "vecs": vecs,
            "wout": wout, "ebias": ebias}


def kernel(x, Wq, gq, bq, Wk, gk, bk, Wv, gv, bv, bias_table, Wout, bout):
    x = np.asarray(x, np.float32)
    if "nc" not in _cache:
        _cache["nc"] = build_nc()
    nc = _cache["nc"]
    shared = _prep_shared(np.asarray(Wq), np.asarray(gq), np.asarray(bq),
                          np.asarray(Wk), np.asarray(gk), np.asarray(bk),
                          np.asarray(Wv), np.asarray(gv), np.asarray(bv),
                          np.asarray(bias_table), np.asarray(Wout), np.asarray(bout))
    in_maps = []
    for b in range(B):
        m = dict(shared)
        m["x"] = np.ascontiguousarray(
            x[b].reshape(NCHUNK, P, IH, IW).astype(ml_dtypes.bfloat16))
        in_maps.append(m)
    _cache["last_in_maps"] = in_maps
    res = bass_utils.run_bass_kernel_spmd(nc, in_maps, core_ids=list(range(B)))
    out = np.stack([r["out"] for r in res.results])          # [B, 128, 2, 1024]
    out = out.transpose(0, 2, 1, 3).reshape(B, C, IH, IW)
    return np.ascontiguousarray(out.astype(np.float32))


if __name__ == "__main__":
    rng = np.random.default_rng(0)
    inputs = {
        'x': rng.standard_normal((B, C, IH, IW), dtype=np.float32),
        'Wq': (rng.standard_normal((C, C, 3, 3)) * 0.02).astype(np.float32),
        'gq': np.ones(C, np.float32), 'bq': np.zeros(C, np.float32),
        'Wk': (rng.standard_normal((C, C, 3, 3)) * 0.02).astype(np.float32),
        'gk': np.ones(C, np.float32), 'bk': np.zeros(C, np.float32),
        'Wv': (rng.standard_normal((C, C, 3, 3)) * 0.02).astype(np.float32),
        'gv': np.ones(C, np.float32), 'bv': np.zeros(C, np.float32),
        'bias_table': (rng.standard_normal(((2 * IH - 1) * (2 * IW - 1), HEADS)) * 0.02).astype(np.float32),
        'Wout': (rng.standard_normal((C, C, 1, 1)) * 0.02).astype(np.float32),
        'bout': np.zeros(C, np.float32),
    }
    out = kernel(**inputs)
    print("out", out.shape, out.dtype, np.abs(out).max())
